# revision 51
# baseline (speedup 1.0000x reference)
"""Baichuan attention decode step on 8 Trainium2 NeuronCores (Bass/Tile).

Head-sharded tensor parallel: 40 heads -> 5 heads per core.
Per core:
  - QKV projection computed TRANSPOSED (qkvT[d, m-tile, tok]): W chunks
    are the PE's stationary operand (bf16 LDWEIGHTS ingests ~2x the rhs
    stream rate) and hsT streams 8 columns. qT / kT_new land directly in
    the dim-major layout the score matmuls need — no PSUM cast or
    transpose on the critical path. (q rows pre-scaled by 1/sqrt(128).)
  - k-cache update: kT_new columns inserted into a small dedicated
    kc_ins tile holding only the affected position chunks, so the DVE
    inserts never gate the main kcT stream's score matmuls.
  - scores (transposed): sT[pos,tok] per head via PE, + mask, exp on ACT
  - softmax denominator via ones-matmul + strided DVE reduce; the
    division is applied per head to the attention output
    (scale-after-matmul), pipelined under the next head's chain
  - v-cache update: stale rows pre-zeroed on host, plus a rank-8
    correction matmul with the new v rows (recovered row-major from
    qkvT by 5 tiny PE transposes against eye128)
  - o_proj partial: out[8,5120] = attn[8,640] @ W_o[:,own_cols].T,
    piece-pipelined against the 5 W_o piece DMAs, stored per piece
Host sums the 8 partial outputs (the "all-reduce").

The kernel is HBM-bandwidth bound (W_pack shard 1920x5120 + W_o shard
5120x640 + k/v cache shards dominate), so all streamed tensors are cast
to bf16 on the host: ~31.7 MB/core vs ~63.5 MB in fp32. Matmuls run
bf16 x bf16 with fp32 PSUM accumulation; softmax logits/denominators
stay fp32 on-chip. Measured output rel err 8.5e-3 (gate 2e-2).

PSUM accumulation chains are kept sequence-contiguous per bank
(interleaved chains in one bank corrupt each other on HW): the QKV
m-tile chains run mt-outer/kc-inner per W_pack DMA group, with group
partials summed on the DVE into an SBUF fp32 accumulator.

Only the 8 mask rows at input_pos are shipped to the device (gathered on
host), and input_pos is baked into the compiled program (recompiled per
distinct input_pos, cached).
"""

import os
import sys
import math
from contextlib import ExitStack

import numpy as np
import ml_dtypes

for _p in ("/opt/trn_rl_repo", "/opt/trn_rl_repo/concourse"):
    if os.path.isdir(_p) and _p not in sys.path:
        sys.path.insert(0, _p)

import concourse.tile as tile  # noqa: E402
from concourse import bacc, mybir  # noqa: E402
from concourse.bass_utils import run_bass_kernel_spmd  # noqa: E402

F32 = mybir.dt.float32
BF16 = mybir.dt.bfloat16
NP_BF16 = ml_dtypes.bfloat16

HIDDEN = 5120
NH = 40
HD = 128
L = 2048
Q = 8
NCORES = 8
HPC = NH // NCORES          # 5 heads per core
KC = HIDDEN // 128          # 40 contraction chunks
MQKV = 3 * HPC * HD         # 1920 qkv output dim per core
NPOS = L // 128             # 16 position chunks
WP_G = 4                    # wpT k-chunks per DMA group
WO_P = 1024                 # o_proj N-piece size

_CACHE = {}


def _build_program(pos, winners):
    """Build the SPMD Bass program with input_pos baked in.

    pos: list of 8 ints. winners: list of bools (True if token t's cache
    write survives, i.e. it is the last occurrence of that position).
    """
    nc = bacc.Bacc("TRN2", target_bir_lowering=False, debug=False)

    hsT_d = nc.dram_tensor("hsT", [128, KC, Q], BF16, kind="ExternalInput")
    wpT_d = nc.dram_tensor("wpT", [128, KC, MQKV], BF16, kind="ExternalInput")
    kcT_d = nc.dram_tensor("kcT", [128, HPC, L], BF16, kind="ExternalInput")
    vc_d = nc.dram_tensor("vc", [128, HPC, NPOS, HD], BF16, kind="ExternalInput")
    maskT_d = nc.dram_tensor("maskT", [128, HPC, NPOS, Q], F32, kind="ExternalInput")
    maskN_d = nc.dram_tensor("maskN", [Q, HPC, Q], F32, kind="ExternalInput")
    n_ins = len({p // 128 for p in pos})
    kcins_d = nc.dram_tensor("kcins", [128, HPC, n_ins * 128], BF16, kind="ExternalInput")
    wo_d = nc.dram_tensor("wo", [128, HIDDEN // WO_P, HPC, WO_P], BF16, kind="ExternalInput")
    ones_d = nc.dram_tensor("ones", [1, 128], F32, kind="ExternalInput")
    onesr_d = nc.dram_tensor("onesr", [128, 1], BF16, kind="ExternalInput")
    eye128_d = nc.dram_tensor("eye128", [128, 128], BF16, kind="ExternalInput")
    out_d = nc.dram_tensor("out", [Q, HIDDEN], F32, kind="ExternalOutput")

    with tile.TileContext(nc) as tc, ExitStack() as ctx:
        sb = ctx.enter_context(tc.tile_pool(name="sb", bufs=1))
        ps = ctx.enter_context(tc.tile_pool(name="ps", bufs=1, space="PSUM"))

        # ---- DMA issue order == HWDGE ring order (FIFO): a big W_pack
        # group first (saturate the ring from the first descriptor), hsT
        # right behind it (PE's first dependency), then the rest of the
        # W_pack stream, with the tiny consts slipped in just before the
        # last group (first needed right after QKV). Then masks/caches
        # (needed at scores), then W_o (needed last, into its own slot
        # so its DMA is never gated on compute). ----
        groups = [WP_G] * (KC // WP_G)
        wp_tiles = []
        # hsT rides the (near-idle) scalar HWDGE ring: arrives during wp
        # group 0's stream and costs the sync ring no boundary slot
        hsT = sb.tile([128, KC, Q], BF16, tag="hsT")
        nc.scalar.dma_start(hsT[:], hsT_d.ap())
        g0 = 0
        for gi, gn in enumerate(groups):
            wp = sb.tile([128, WP_G, MQKV], BF16, tag="wstream", bufs=4)
            nc.sync.dma_start(wp[:, 0:gn, :], wpT_d.ap()[:, g0:g0 + gn, :])
            wp_tiles.append((g0, gn, wp))
            g0 += gn
        # consts ride the (otherwise idle) scalar HWDGE ring so they
        # never hole the main sync stream
        ones_row = sb.tile([1, 128], F32, tag="ones")
        nc.scalar.dma_start(ones_row[:], ones_d.ap())
        ones_r = sb.tile([128, 1], BF16, tag="onesr")
        nc.scalar.dma_start(ones_r[:], onesr_d.ap())
        eye128 = sb.tile([128, 128], BF16, tag="eye128")
        nc.scalar.dma_start(eye128[:], eye128_d.ap())
        # masks + kc_ins also ride the scalar ring: they're small, needed
        # only at the score phase, and each would otherwise cost the sync
        # ring a boundary slot between the wp stream and the caches
        maskT = sb.tile([128, HPC, NPOS, Q], F32, tag="maskT")
        nc.scalar.dma_start(maskT[:], maskT_d.ap())
        maskN = sb.tile([Q, HPC, Q], F32, tag="maskN")
        nc.scalar.dma_start(maskN[:], maskN_d.ap())
        # position chunks receiving k-cache inserts get their own small
        # tile (DMA'd first, from a contiguous host copy): the DVE insert
        # copies then gate only the scores of these chunks, not the whole
        # kcT stream (deps are tile-granular).
        ins_cjs = sorted({p // 128 for p in pos})
        ins_idx = {cj: i for i, cj in enumerate(ins_cjs)}
        kc_ins = sb.tile([128, HPC, len(ins_cjs) * 128], BF16, tag="kcins")
        nc.scalar.dma_start(kc_ins[:], kcins_d.ap())
        # each sync-ring DMA costs ~0.45us of boundary time on top of its
        # streaming time, so caches go as ONE DMA each: kcT first (gates
        # the score chains), vc second (attention needs all heads anyway)
        kcT = sb.tile([128, HPC, L], BF16, tag="kcT")
        vc = sb.tile([128, HPC, NPOS, HD], BF16, tag="vc")
        nc.sync.dma_start(kcT[:], kcT_d.ap())
        # vc in two halves: attention h0-2 starts ~2.5us earlier, and the
        # shorter PE idle before it stays under the ~3.4us HAM re-throttle
        # window (cold-clock o_proj start otherwise)
        nc.sync.dma_start(vc[:, 0:3], vc_d.ap()[:, 0:3])
        nc.sync.dma_start(vc[:, 3:HPC], vc_d.ap()[:, 3:HPC])
        # W_o: one dedicated tile per piece (precise per-piece deps for
        # o_proj), streamed right behind the caches
        wo_sbs = []
        NP_PIECE = HIDDEN // WO_P
        for piece in range(NP_PIECE):
            w = sb.tile([128, HPC, WO_P], BF16, tag=f"wo{piece}")
            if piece == NP_PIECE - 1:
                # last piece in two halves: the compute exposed after the
                # stream ends is one 512-slice, not a whole piece
                nc.sync.dma_start(w[:, :, 0:512], wo_d.ap()[:, piece, :, 0:512])
                nc.sync.dma_start(w[:, :, 512:WO_P], wo_d.ap()[:, piece, :, 512:WO_P])
            else:
                nc.sync.dma_start(w[:], wo_d.ap()[:, piece, :, :])
            wo_sbs.append(w)

        # ---- QKV projection, computed TRANSPOSED: qkvT[d, mt, t] ----
        # W chunks are the stationary operand (bf16 LDWEIGHTS ingests ~2
        # cols/cycle, twice the rhs stream rate), hsT streams 8 cols.
        # m-tiles 0-4 = qT heads, 5-9 = kT_new heads, 10-14 = vT_new heads
        # -> qT/kT land directly in scores layout, no psum cast/transpose
        # on the critical path.
        # PSUM accumulation chains must be SEQUENCE-contiguous: interleaved
        # chains in one bank corrupt each other (measured 0.46 rel err), so
        # each group computes chain-sequential partials (mt outer, kc
        # inner) and groups are summed on the DVE into an SBUF fp32 acc.
        NMT = 3 * HPC
        qacc = sb.tile([128, NMT, Q], F32, tag="qacc")
        qkvT = sb.tile([128, NMT, Q], BF16, tag="qkvT")
        ngroups = len(wp_tiles)
        for gi, (g0, gn, wp) in enumerate(wp_tiles):
            pg = ps.tile([128, NMT, Q], F32, tag="QS", bufs=2)
            for mt in range(NMT):
                for i in range(gn):
                    nc.tensor.matmul(
                        pg[:, mt, :],
                        wp[:, i, mt * HD:(mt + 1) * HD],
                        hsT[:, g0 + i, :],
                        start=(i == 0),
                        stop=(i == gn - 1),
                    )
            if gi == 0:
                nc.vector.tensor_copy(qacc[:], pg[:])
            elif gi < ngroups - 1:
                nc.vector.tensor_add(qacc[:], qacc[:], pg[:])
            else:
                nc.vector.tensor_add(qkvT[:], qacc[:], pg[:])
        qT = qkvT[:, 0:HPC, :]
        ktn = qkvT[:, HPC:2 * HPC, :]

        # ---- v_new rows (row-major, for the rank-8 correction) ----
        vnew = sb.tile([Q, HPC, HD], BF16, tag="vnew")
        for h in range(HPC):
            tv = ps.tile([Q, HD], BF16, tag="S1", bufs=2)
            nc.tensor.transpose(tv[:], qkvT[:, 2 * HPC + h, :], eye128[:])
            nc.vector.tensor_copy(vnew[0:Q, h, :], tv[:])

        # ---- k-cache insert (winner tokens only, into kc_ins) ----
        for t in range(Q):
            if winners[t]:
                p = pos[t]
                col = ins_idx[p // 128] * 128 + p % 128
                nc.vector.tensor_copy(kc_ins[:, :, col], ktn[:, :, t])

        # ---- correction rhs: corr[j, h, t] = exp(k_new_j . q_t + mask[j, t])
        # (duplicate losers get mask -1e30 -> exp == 0; winners match the
        #  main score path at the inserted cache columns)
        corr_ps = ps.tile([Q, HPC, Q], F32, tag="S1", bufs=2)
        for h in range(HPC):
            nc.tensor.matmul(
                corr_ps[0:Q, h, :], ktn[:, h, :], qT[:, h, :], start=True, stop=True
            )
        corr_s = sb.tile([Q, HPC, Q], F32, tag="corrs")
        nc.vector.tensor_add(corr_s[:], corr_ps[:], maskN[:])
        corr = sb.tile([Q, HPC, Q], BF16, tag="corr")
        nc.scalar.activation(corr[:], corr_s[:], mybir.ActivationFunctionType.Exp)

        # ---- per-head: scores (transposed) + mask + exp, denominator,
        # attention — interleaved so head h's chain runs as soon as its
        # own kcT/vc slices land ----
        scT = sb.tile([128, HPC, NPOS, Q], F32, tag="scT")
        expT = sb.tile([128, HPC, NPOS, Q], BF16, tag="expT")
        ps_at = ps.tile([128, HPC, Q], F32, tag="AT", bufs=1)
        sums = sb.tile([1, HPC, Q], F32, tag="sums")
        recip = sb.tile([1, HPC, Q], F32, tag="recip")
        attn = sb.tile([128, HPC * Q], BF16, tag="attn")

        for h in range(HPC):
            ps_sc = ps.tile([128, NPOS, Q], F32, tag="QS", bufs=2)
            for cj in range(NPOS):
                if cj in ins_idx:
                    ksrc = kc_ins[:, h, ins_idx[cj] * 128:(ins_idx[cj] + 1) * 128]
                else:
                    ksrc = kcT[:, h, cj * 128:(cj + 1) * 128]
                nc.tensor.matmul(
                    ps_sc[:, cj, :],
                    ksrc,
                    qkvT[:, h, :],
                    start=True,
                    stop=True,
                )
            nc.vector.tensor_add(scT[:, h], ps_sc[:], maskT[:, h])
            nc.scalar.activation(expT[:, h], scT[:, h], mybir.ActivationFunctionType.Exp)
            # denominator partial sums over the partition (position) axis
            ps_sum = ps.tile([1, NPOS, Q], F32, tag="PO", bufs=2)
            nc.tensor.matmul(
                ps_sum[0:1],
                ones_r[:, 0:1],
                expT[:, h].rearrange("p c t -> p (c t)"),
                start=True,
                stop=True,
            )
            nc.vector.tensor_reduce(
                sums[0:1, h, :],
                ps_sum.rearrange("p c t -> p t c"),
                axis=mybir.AxisListType.X,
                op=mybir.AluOpType.add,
            )
            # reciprocal early (DVE, overlaps later heads' chains)
            nc.vector.reciprocal(recip[0:1, h, :], sums[0:1, h, :])

        # attention in its own pass: in PE program order the vc-gated
        # attention matmuls would otherwise sit between scores h and
        # scores h+1 and idle the PE until vc lands
        for h in range(HPC):
            # attention: attnT[d, t] accumulated over position chunks
            for cj in range(NPOS):
                nc.tensor.matmul(
                    ps_at[:, h, :],
                    vc[:, h, cj, :],
                    expT[:, h, cj, :],
                    start=(cj == 0),
                    stop=False,
                )
            # + rank-8 correction with the new v rows
            nc.tensor.matmul(
                ps_at[:, h, :],
                vnew[0:Q, h, :],
                corr[:, h, :],
                start=False,
                stop=True,
            )

        # softmax division AFTER all attention matmuls: keeping the
        # broadcast matmuls out of the head loop stops each head's
        # attention from serializing behind the previous head's DVE
        # softmax tail in PE program order
        for h in range(HPC):
            ps_bch = ps.tile([128, Q], F32, tag="S1", bufs=2)
            nc.tensor.matmul(
                ps_bch[:], ones_row[0:1, :], recip[0:1, h, :], start=True, stop=True
            )
            bch = sb.tile([128, Q], F32, tag="bcsb", bufs=2)
            nc.vector.tensor_copy(bch[:], ps_bch[:])
            nc.vector.tensor_mul(attn[:, h * Q:(h + 1) * Q], ps_at[:, h, :], bch[:])

        # ---- o_proj partial: out[8, 5120], stored piece-by-piece ----
        out_sb = sb.tile([Q, HIDDEN], F32, tag="outsb")
        for piece in range(HIDDEN // WO_P):
            wo = wo_sbs[piece]
            for s in range(WO_P // 512):
                ps_o = ps.tile([Q, 512], F32, tag="PO", bufs=2)
                for h in range(HPC):
                    nc.tensor.matmul(
                        ps_o[0:Q, :],
                        attn[:, h * Q:(h + 1) * Q],
                        wo[:, h, s * 512:(s + 1) * 512],
                        start=(h == 0),
                        stop=(h == HPC - 1),
                    )
                n0 = piece * WO_P + s * 512
                nc.vector.tensor_copy(out_sb[0:Q, n0:n0 + 512], ps_o[:])
                if piece == HIDDEN // WO_P - 1:
                    # last piece stores per 512-slice: final exposure is
                    # one slice's compute + store, not the whole piece
                    nc.sync.dma_start(
                        out_d.ap()[:, n0:n0 + 512], out_sb[0:Q, n0:n0 + 512]
                    )
            if piece < HIDDEN // WO_P - 1:
                p0 = piece * WO_P
                nc.sync.dma_start(
                    out_d.ap()[:, p0:p0 + WO_P], out_sb[0:Q, p0:p0 + WO_P]
                )

    nc.compile()
    return nc


def _get_program(pos, winners):
    key = (tuple(pos), tuple(winners))
    if key not in _CACHE:
        _CACHE[key] = _build_program(pos, winners)
    return _CACHE[key]


def _prep_inputs(input_pos, hidden_states, attention_mask, W_pack, W_o,
                 k_cache, v_cache):
    """Host-side sharding: returns (in_maps, pos, winners)."""
    pos = [int(p) for p in np.asarray(input_pos).reshape(-1)]
    last = {}
    for t, p in enumerate(pos):
        last[p] = t
    winners = [last[p] == t for t, p in enumerate(pos)]

    hs = np.asarray(hidden_states, dtype=np.float32).reshape(Q, HIDDEN)
    # hsT[p, kc, t] = hs[t, kc*128+p]
    hsT = np.ascontiguousarray(
        hs.T.reshape(KC, 128, Q).transpose(1, 0, 2)).astype(NP_BF16)

    Wp = np.asarray(W_pack, dtype=np.float32)
    Wo = np.asarray(W_o, dtype=np.float32)
    kc_all = np.asarray(k_cache, dtype=np.float32)[0]   # [40, 2048, 128]
    vc_all = np.asarray(v_cache, dtype=np.float32)[0]
    mask = np.asarray(attention_mask, dtype=np.float32)
    mrows = mask[:, pos, :]                              # [40, 8, 2048]

    scale = np.float32(1.0 / math.sqrt(HD))
    ones = np.ones((1, 128), dtype=np.float32)
    ones_col = np.ones((128, 1), dtype=NP_BF16)
    eye128 = np.eye(128, dtype=NP_BF16)

    in_maps = []
    for c in range(NCORES):
        r0 = c * HPC * HD
        r1 = (c + 1) * HPC * HD
        wsh = np.concatenate(
            [Wp[r0:r1] * scale, Wp[HIDDEN + r0:HIDDEN + r1],
             Wp[2 * HIDDEN + r0:2 * HIDDEN + r1]], axis=0)   # [1920, 5120]
        # [128 p, 40 kc, 1920 m]
        wpT = np.ascontiguousarray(
            wsh.T.reshape(KC, 128, MQKV).transpose(1, 0, 2)).astype(NP_BF16)
        heads = slice(c * HPC, (c + 1) * HPC)
        # [128 d, 5 h, 2048 pos]
        kcT = np.ascontiguousarray(
            kc_all[heads].transpose(2, 0, 1)).astype(NP_BF16)
        # contiguous copy of the position chunks that receive inserts
        ins_cjs = sorted({p // 128 for p in pos})
        kcins = np.ascontiguousarray(np.concatenate(
            [kcT[:, :, cj * 128:(cj + 1) * 128] for cj in ins_cjs], axis=2))
        vcc = vc_all[heads].reshape(HPC, NPOS, 128, HD).copy()
        for t in range(Q):
            if winners[t]:
                vcc[:, pos[t] // 128, pos[t] % 128, :] = 0.0
        # [128 p, 5 h, 16 c, 128 d]
        vcc = np.ascontiguousarray(vcc.transpose(2, 0, 1, 3)).astype(NP_BF16)
        # maskT[p, h, cj, t] = mrows[own_h, t, cj*128+p]
        mT = np.ascontiguousarray(
            mrows[heads].reshape(HPC, Q, NPOS, 128).transpose(3, 0, 2, 1))
        # maskN[j, h, t] = mrows[own_h, t, pos_j]; -1e30 for duplicate losers
        mN = np.ascontiguousarray(mrows[heads][:, :, pos].transpose(2, 0, 1))
        for j in range(Q):
            if not winners[j]:
                mN[j] = np.float32(-1e30)
        # [128 p, 5 piece, 5 h, 1024 n]
        wo = np.ascontiguousarray(
            Wo[:, r0:r1].reshape(HIDDEN // WO_P, WO_P, HPC, 128)
            .transpose(3, 0, 2, 1)).astype(NP_BF16)
        in_maps.append({
            "hsT": hsT, "wpT": wpT, "kcT": kcT, "kcins": kcins, "vc": vcc,
            "maskT": mT, "maskN": mN, "wo": wo, "ones": ones,
            "eye128": eye128, "onesr": ones_col,
        })
    return in_maps, pos, winners


def kernel(input_pos, hidden_states, attention_mask, W_pack, W_o,
           k_cache, v_cache, _profile=False):
    in_maps, pos, winners = _prep_inputs(
        input_pos, hidden_states, attention_mask, W_pack, W_o, k_cache, v_cache)
    nc = _get_program(pos, winners)
    res = run_bass_kernel_spmd(nc, in_maps, list(range(NCORES)), trace=_profile)
    out = np.zeros((Q, HIDDEN), dtype=np.float64)
    for r in res.results:
        out += r["out"].astype(np.float64)
    full = out.astype(np.float32).reshape(1, Q, HIDDEN)
    if _profile:
        return full, res
    return full


# revision 52
# speedup vs baseline: 1.0036x; 1.0036x over previous
"""Baichuan attention decode step on 8 Trainium2 NeuronCores (Bass/Tile).

Head-sharded tensor parallel: 40 heads -> 5 heads per core.
Per core:
  - QKV projection computed TRANSPOSED (qkvT[d, m-tile, tok]): W chunks
    are the PE's stationary operand (bf16 LDWEIGHTS ingests ~2x the rhs
    stream rate) and hsT streams 8 columns. qT / kT_new land directly in
    the dim-major layout the score matmuls need — no PSUM cast or
    transpose on the critical path. (q rows pre-scaled by 1/sqrt(128).)
  - k-cache update: kT_new columns inserted into a small dedicated
    kc_ins tile holding only the affected position chunks, so the DVE
    inserts never gate the main kcT stream's score matmuls.
  - scores (transposed): sT[pos,tok] per head via PE, + mask, exp on ACT
  - softmax denominator via ones-matmul + strided DVE reduce; the
    division is applied per head to the attention output
    (scale-after-matmul), pipelined under the next head's chain
  - v-cache update: stale rows pre-zeroed on host, plus a rank-8
    correction matmul with the new v rows (recovered row-major from
    qkvT by 5 tiny PE transposes against eye128)
  - o_proj partial: out[8,5120] = attn[8,640] @ W_o[:,own_cols].T,
    piece-pipelined against the 5 W_o piece DMAs, stored per piece
Host sums the 8 partial outputs (the "all-reduce").

The kernel is HBM-bandwidth bound (W_pack shard 1920x5120 + W_o shard
5120x640 + k/v cache shards dominate), so all streamed tensors are cast
to bf16 on the host: ~31.7 MB/core vs ~63.5 MB in fp32. Matmuls run
bf16 x bf16 with fp32 PSUM accumulation; softmax logits/denominators
stay fp32 on-chip. Measured output rel err 8.5e-3 (gate 2e-2).

PSUM accumulation chains are kept sequence-contiguous per bank
(interleaved chains in one bank corrupt each other on HW): the QKV
m-tile chains run mt-outer/kc-inner per W_pack DMA group, with group
partials summed on the DVE into an SBUF fp32 accumulator.

Only the 8 mask rows at input_pos are shipped to the device (gathered on
host), and input_pos is baked into the compiled program (recompiled per
distinct input_pos, cached).
"""

import os
import sys
import math
from contextlib import ExitStack

import numpy as np
import ml_dtypes

for _p in ("/opt/trn_rl_repo", "/opt/trn_rl_repo/concourse"):
    if os.path.isdir(_p) and _p not in sys.path:
        sys.path.insert(0, _p)

import concourse.tile as tile  # noqa: E402
from concourse import bacc, mybir  # noqa: E402
from concourse.bass_utils import run_bass_kernel_spmd  # noqa: E402

F32 = mybir.dt.float32
BF16 = mybir.dt.bfloat16
NP_BF16 = ml_dtypes.bfloat16

HIDDEN = 5120
NH = 40
HD = 128
L = 2048
Q = 8
NCORES = 8
HPC = NH // NCORES          # 5 heads per core
KC = HIDDEN // 128          # 40 contraction chunks
MQKV = 3 * HPC * HD         # 1920 qkv output dim per core
NPOS = L // 128             # 16 position chunks
WP_G = 4                    # wpT k-chunks per DMA group
WO_P = 1024                 # o_proj N-piece size

_CACHE = {}


def _build_program(pos, winners):
    """Build the SPMD Bass program with input_pos baked in.

    pos: list of 8 ints. winners: list of bools (True if token t's cache
    write survives, i.e. it is the last occurrence of that position).
    """
    nc = bacc.Bacc("TRN2", target_bir_lowering=False, debug=False)

    hsT_d = nc.dram_tensor("hsT", [128, KC, Q], BF16, kind="ExternalInput")
    wpT_d = nc.dram_tensor("wpT", [128, KC, MQKV], BF16, kind="ExternalInput")
    kcT_d = nc.dram_tensor("kcT", [128, HPC, L], BF16, kind="ExternalInput")
    vc_d = nc.dram_tensor("vc", [128, HPC, NPOS, HD], BF16, kind="ExternalInput")
    maskT_d = nc.dram_tensor("maskT", [128, HPC, NPOS, Q], BF16, kind="ExternalInput")
    maskN_d = nc.dram_tensor("maskN", [Q, HPC, Q], F32, kind="ExternalInput")
    n_ins = len({p // 128 for p in pos})
    kcins_d = nc.dram_tensor("kcins", [128, HPC, n_ins * 128], BF16, kind="ExternalInput")
    wo_d = nc.dram_tensor("wo", [128, HIDDEN // WO_P, HPC, WO_P], BF16, kind="ExternalInput")
    ones_d = nc.dram_tensor("ones", [1, 128], F32, kind="ExternalInput")
    onesr_d = nc.dram_tensor("onesr", [128, 1], BF16, kind="ExternalInput")
    eye128_d = nc.dram_tensor("eye128", [128, 128], BF16, kind="ExternalInput")
    out_d = nc.dram_tensor("out", [Q, HIDDEN], BF16, kind="ExternalOutput")

    with tile.TileContext(nc) as tc, ExitStack() as ctx:
        sb = ctx.enter_context(tc.tile_pool(name="sb", bufs=1))
        ps = ctx.enter_context(tc.tile_pool(name="ps", bufs=1, space="PSUM"))

        # ---- DMA issue order == HWDGE ring order (FIFO): a big W_pack
        # group first (saturate the ring from the first descriptor), hsT
        # right behind it (PE's first dependency), then the rest of the
        # W_pack stream, with the tiny consts slipped in just before the
        # last group (first needed right after QKV). Then masks/caches
        # (needed at scores), then W_o (needed last, into its own slot
        # so its DMA is never gated on compute). ----
        groups = [WP_G] * (KC // WP_G)
        wp_tiles = []
        # hsT rides the (near-idle) scalar HWDGE ring: arrives during wp
        # group 0's stream and costs the sync ring no boundary slot
        hsT = sb.tile([128, KC, Q], BF16, tag="hsT")
        nc.scalar.dma_start(hsT[:], hsT_d.ap())
        g0 = 0
        for gi, gn in enumerate(groups):
            wp = sb.tile([128, WP_G, MQKV], BF16, tag="wstream", bufs=4)
            nc.sync.dma_start(wp[:, 0:gn, :], wpT_d.ap()[:, g0:g0 + gn, :])
            wp_tiles.append((g0, gn, wp))
            g0 += gn
        # consts ride the (otherwise idle) scalar HWDGE ring so they
        # never hole the main sync stream
        ones_row = sb.tile([1, 128], F32, tag="ones")
        nc.scalar.dma_start(ones_row[:], ones_d.ap())
        ones_r = sb.tile([128, 1], BF16, tag="onesr")
        nc.scalar.dma_start(ones_r[:], onesr_d.ap())
        eye128 = sb.tile([128, 128], BF16, tag="eye128")
        nc.scalar.dma_start(eye128[:], eye128_d.ap())
        # masks + kc_ins also ride the scalar ring: they're small, needed
        # only at the score phase, and each would otherwise cost the sync
        # ring a boundary slot between the wp stream and the caches
        maskT = sb.tile([128, HPC, NPOS, Q], BF16, tag="maskT")
        nc.scalar.dma_start(maskT[:], maskT_d.ap())
        maskN = sb.tile([Q, HPC, Q], F32, tag="maskN")
        nc.scalar.dma_start(maskN[:], maskN_d.ap())
        # position chunks receiving k-cache inserts get their own small
        # tile (DMA'd first, from a contiguous host copy): the DVE insert
        # copies then gate only the scores of these chunks, not the whole
        # kcT stream (deps are tile-granular).
        ins_cjs = sorted({p // 128 for p in pos})
        ins_idx = {cj: i for i, cj in enumerate(ins_cjs)}
        kc_ins = sb.tile([128, HPC, len(ins_cjs) * 128], BF16, tag="kcins")
        nc.scalar.dma_start(kc_ins[:], kcins_d.ap())
        # each sync-ring DMA costs ~0.45us of boundary time on top of its
        # streaming time, so caches go as ONE DMA each: kcT first (gates
        # the score chains), vc second (attention needs all heads anyway)
        kcT = sb.tile([128, HPC, L], BF16, tag="kcT")
        vc = sb.tile([128, HPC, NPOS, HD], BF16, tag="vc")
        nc.sync.dma_start(kcT[:], kcT_d.ap())
        # vc in two halves: attention h0-2 starts ~2.5us earlier, and the
        # shorter PE idle before it stays under the ~3.4us HAM re-throttle
        # window (cold-clock o_proj start otherwise)
        nc.sync.dma_start(vc[:, 0:3], vc_d.ap()[:, 0:3])
        nc.sync.dma_start(vc[:, 3:HPC], vc_d.ap()[:, 3:HPC])
        # W_o: one dedicated tile per piece (precise per-piece deps for
        # o_proj), streamed right behind the caches
        wo_sbs = []
        NP_PIECE = HIDDEN // WO_P
        for piece in range(NP_PIECE):
            w = sb.tile([128, HPC, WO_P], BF16, tag=f"wo{piece}")
            if piece == NP_PIECE - 1:
                # last piece in two halves: the compute exposed after the
                # stream ends is one 512-slice, not a whole piece
                nc.sync.dma_start(w[:, :, 0:512], wo_d.ap()[:, piece, :, 0:512])
                nc.sync.dma_start(w[:, :, 512:WO_P], wo_d.ap()[:, piece, :, 512:WO_P])
            else:
                nc.sync.dma_start(w[:], wo_d.ap()[:, piece, :, :])
            wo_sbs.append(w)

        # ---- QKV projection, computed TRANSPOSED: qkvT[d, mt, t] ----
        # W chunks are the stationary operand (bf16 LDWEIGHTS ingests ~2
        # cols/cycle, twice the rhs stream rate), hsT streams 8 cols.
        # m-tiles 0-4 = qT heads, 5-9 = kT_new heads, 10-14 = vT_new heads
        # -> qT/kT land directly in scores layout, no psum cast/transpose
        # on the critical path.
        # PSUM accumulation chains must be SEQUENCE-contiguous: interleaved
        # chains in one bank corrupt each other (measured 0.46 rel err), so
        # each group computes chain-sequential partials (mt outer, kc
        # inner) and groups are summed on the DVE into an SBUF fp32 acc.
        NMT = 3 * HPC
        qacc = sb.tile([128, NMT, Q], F32, tag="qacc")
        qkvT = sb.tile([128, NMT, Q], BF16, tag="qkvT")
        ngroups = len(wp_tiles)
        for gi, (g0, gn, wp) in enumerate(wp_tiles):
            pg = ps.tile([128, NMT, Q], F32, tag="QS", bufs=2)
            for mt in range(NMT):
                for i in range(gn):
                    nc.tensor.matmul(
                        pg[:, mt, :],
                        wp[:, i, mt * HD:(mt + 1) * HD],
                        hsT[:, g0 + i, :],
                        start=(i == 0),
                        stop=(i == gn - 1),
                    )
            if gi == 0:
                nc.vector.tensor_copy(qacc[:], pg[:])
            elif gi < ngroups - 1:
                nc.vector.tensor_add(qacc[:], qacc[:], pg[:])
            else:
                nc.vector.tensor_add(qkvT[:], qacc[:], pg[:])
        qT = qkvT[:, 0:HPC, :]
        ktn = qkvT[:, HPC:2 * HPC, :]

        # ---- v_new rows (row-major, for the rank-8 correction) ----
        vnew = sb.tile([Q, HPC, HD], BF16, tag="vnew")
        for h in range(HPC):
            tv = ps.tile([Q, HD], BF16, tag="S1", bufs=2)
            nc.tensor.transpose(tv[:], qkvT[:, 2 * HPC + h, :], eye128[:])
            nc.vector.tensor_copy(vnew[0:Q, h, :], tv[:])

        # ---- k-cache insert (winner tokens only, into kc_ins) ----
        for t in range(Q):
            if winners[t]:
                p = pos[t]
                col = ins_idx[p // 128] * 128 + p % 128
                nc.vector.tensor_copy(kc_ins[:, :, col], ktn[:, :, t])

        # ---- correction rhs: corr[j, h, t] = exp(k_new_j . q_t + mask[j, t])
        # (duplicate losers get mask -1e30 -> exp == 0; winners match the
        #  main score path at the inserted cache columns)
        corr_ps = ps.tile([Q, HPC, Q], F32, tag="S1", bufs=2)
        for h in range(HPC):
            nc.tensor.matmul(
                corr_ps[0:Q, h, :], ktn[:, h, :], qT[:, h, :], start=True, stop=True
            )
        corr_s = sb.tile([Q, HPC, Q], F32, tag="corrs")
        nc.vector.tensor_add(corr_s[:], corr_ps[:], maskN[:])
        corr = sb.tile([Q, HPC, Q], BF16, tag="corr")
        nc.scalar.activation(corr[:], corr_s[:], mybir.ActivationFunctionType.Exp)

        # ---- per-head: scores (transposed) + mask + exp, denominator,
        # attention — interleaved so head h's chain runs as soon as its
        # own kcT/vc slices land ----
        scT = sb.tile([128, HPC, NPOS, Q], F32, tag="scT")
        expT = sb.tile([128, HPC, NPOS, Q], BF16, tag="expT")
        ps_at = ps.tile([128, HPC, Q], F32, tag="AT", bufs=1)
        sums = sb.tile([1, HPC, Q], F32, tag="sums")
        recip = sb.tile([1, HPC, Q], F32, tag="recip")
        attn = sb.tile([128, HPC * Q], BF16, tag="attn")

        for h in range(HPC):
            ps_sc = ps.tile([128, NPOS, Q], F32, tag="QS", bufs=2)
            for cj in range(NPOS):
                if cj in ins_idx:
                    ksrc = kc_ins[:, h, ins_idx[cj] * 128:(ins_idx[cj] + 1) * 128]
                else:
                    ksrc = kcT[:, h, cj * 128:(cj + 1) * 128]
                nc.tensor.matmul(
                    ps_sc[:, cj, :],
                    ksrc,
                    qkvT[:, h, :],
                    start=True,
                    stop=True,
                )
            nc.vector.tensor_add(scT[:, h], ps_sc[:], maskT[:, h])
            nc.scalar.activation(expT[:, h], scT[:, h], mybir.ActivationFunctionType.Exp)
            # denominator partial sums over the partition (position) axis
            ps_sum = ps.tile([1, NPOS, Q], F32, tag="PO", bufs=2)
            nc.tensor.matmul(
                ps_sum[0:1],
                ones_r[:, 0:1],
                expT[:, h].rearrange("p c t -> p (c t)"),
                start=True,
                stop=True,
            )
            nc.vector.tensor_reduce(
                sums[0:1, h, :],
                ps_sum.rearrange("p c t -> p t c"),
                axis=mybir.AxisListType.X,
                op=mybir.AluOpType.add,
            )
            # reciprocal early (DVE, overlaps later heads' chains)
            nc.vector.reciprocal(recip[0:1, h, :], sums[0:1, h, :])

        # attention in its own pass: in PE program order the vc-gated
        # attention matmuls would otherwise sit between scores h and
        # scores h+1 and idle the PE until vc lands
        for h in range(HPC):
            # attention: attnT[d, t] accumulated over position chunks
            for cj in range(NPOS):
                nc.tensor.matmul(
                    ps_at[:, h, :],
                    vc[:, h, cj, :],
                    expT[:, h, cj, :],
                    start=(cj == 0),
                    stop=False,
                )
            # + rank-8 correction with the new v rows
            nc.tensor.matmul(
                ps_at[:, h, :],
                vnew[0:Q, h, :],
                corr[:, h, :],
                start=False,
                stop=True,
            )

        # softmax division AFTER all attention matmuls: keeping the
        # broadcast matmuls out of the head loop stops each head's
        # attention from serializing behind the previous head's DVE
        # softmax tail in PE program order
        for h in range(HPC):
            ps_bch = ps.tile([128, Q], F32, tag="S1", bufs=2)
            nc.tensor.matmul(
                ps_bch[:], ones_row[0:1, :], recip[0:1, h, :], start=True, stop=True
            )
            bch = sb.tile([128, Q], F32, tag="bcsb", bufs=2)
            nc.vector.tensor_copy(bch[:], ps_bch[:])
            nc.vector.tensor_mul(attn[:, h * Q:(h + 1) * Q], ps_at[:, h, :], bch[:])

        # ---- o_proj partial: out[8, 5120], stored piece-by-piece ----
        out_sb = sb.tile([Q, HIDDEN], BF16, tag="outsb")
        for piece in range(HIDDEN // WO_P):
            wo = wo_sbs[piece]
            for s in range(WO_P // 512):
                ps_o = ps.tile([Q, 512], F32, tag="PO", bufs=2)
                for h in range(HPC):
                    nc.tensor.matmul(
                        ps_o[0:Q, :],
                        attn[:, h * Q:(h + 1) * Q],
                        wo[:, h, s * 512:(s + 1) * 512],
                        start=(h == 0),
                        stop=(h == HPC - 1),
                    )
                n0 = piece * WO_P + s * 512
                nc.vector.tensor_copy(out_sb[0:Q, n0:n0 + 512], ps_o[:])
                if piece == HIDDEN // WO_P - 1:
                    # last piece stores per 512-slice: final exposure is
                    # one slice's compute + store, not the whole piece
                    nc.sync.dma_start(
                        out_d.ap()[:, n0:n0 + 512], out_sb[0:Q, n0:n0 + 512]
                    )
            if piece < HIDDEN // WO_P - 1:
                p0 = piece * WO_P
                nc.sync.dma_start(
                    out_d.ap()[:, p0:p0 + WO_P], out_sb[0:Q, p0:p0 + WO_P]
                )

    nc.compile()
    return nc


def _get_program(pos, winners):
    key = (tuple(pos), tuple(winners))
    if key not in _CACHE:
        _CACHE[key] = _build_program(pos, winners)
    return _CACHE[key]


def _prep_inputs(input_pos, hidden_states, attention_mask, W_pack, W_o,
                 k_cache, v_cache):
    """Host-side sharding: returns (in_maps, pos, winners)."""
    pos = [int(p) for p in np.asarray(input_pos).reshape(-1)]
    last = {}
    for t, p in enumerate(pos):
        last[p] = t
    winners = [last[p] == t for t, p in enumerate(pos)]

    hs = np.asarray(hidden_states, dtype=np.float32).reshape(Q, HIDDEN)
    # hsT[p, kc, t] = hs[t, kc*128+p]
    hsT = np.ascontiguousarray(
        hs.T.reshape(KC, 128, Q).transpose(1, 0, 2)).astype(NP_BF16)

    Wp = np.asarray(W_pack, dtype=np.float32)
    Wo = np.asarray(W_o, dtype=np.float32)
    kc_all = np.asarray(k_cache, dtype=np.float32)[0]   # [40, 2048, 128]
    vc_all = np.asarray(v_cache, dtype=np.float32)[0]
    mask = np.asarray(attention_mask, dtype=np.float32)
    mrows = mask[:, pos, :]                              # [40, 8, 2048]

    scale = np.float32(1.0 / math.sqrt(HD))
    ones = np.ones((1, 128), dtype=np.float32)
    ones_col = np.ones((128, 1), dtype=NP_BF16)
    eye128 = np.eye(128, dtype=NP_BF16)

    in_maps = []
    for c in range(NCORES):
        r0 = c * HPC * HD
        r1 = (c + 1) * HPC * HD
        wsh = np.concatenate(
            [Wp[r0:r1] * scale, Wp[HIDDEN + r0:HIDDEN + r1],
             Wp[2 * HIDDEN + r0:2 * HIDDEN + r1]], axis=0)   # [1920, 5120]
        # [128 p, 40 kc, 1920 m]
        wpT = np.ascontiguousarray(
            wsh.T.reshape(KC, 128, MQKV).transpose(1, 0, 2)).astype(NP_BF16)
        heads = slice(c * HPC, (c + 1) * HPC)
        # [128 d, 5 h, 2048 pos]
        kcT = np.ascontiguousarray(
            kc_all[heads].transpose(2, 0, 1)).astype(NP_BF16)
        # contiguous copy of the position chunks that receive inserts
        ins_cjs = sorted({p // 128 for p in pos})
        kcins = np.ascontiguousarray(np.concatenate(
            [kcT[:, :, cj * 128:(cj + 1) * 128] for cj in ins_cjs], axis=2))
        vcc = vc_all[heads].reshape(HPC, NPOS, 128, HD).copy()
        for t in range(Q):
            if winners[t]:
                vcc[:, pos[t] // 128, pos[t] % 128, :] = 0.0
        # [128 p, 5 h, 16 c, 128 d]
        vcc = np.ascontiguousarray(vcc.transpose(2, 0, 1, 3)).astype(NP_BF16)
        # maskT[p, h, cj, t] = mrows[own_h, t, cj*128+p]
        mT = np.ascontiguousarray(
            mrows[heads].reshape(HPC, Q, NPOS, 128).transpose(3, 0, 2, 1)).astype(NP_BF16)
        # maskN[j, h, t] = mrows[own_h, t, pos_j]; -1e30 for duplicate losers
        mN = np.ascontiguousarray(mrows[heads][:, :, pos].transpose(2, 0, 1))
        for j in range(Q):
            if not winners[j]:
                mN[j] = np.float32(-1e30)
        # [128 p, 5 piece, 5 h, 1024 n]
        wo = np.ascontiguousarray(
            Wo[:, r0:r1].reshape(HIDDEN // WO_P, WO_P, HPC, 128)
            .transpose(3, 0, 2, 1)).astype(NP_BF16)
        in_maps.append({
            "hsT": hsT, "wpT": wpT, "kcT": kcT, "kcins": kcins, "vc": vcc,
            "maskT": mT, "maskN": mN, "wo": wo, "ones": ones,
            "eye128": eye128, "onesr": ones_col,
        })
    return in_maps, pos, winners


def kernel(input_pos, hidden_states, attention_mask, W_pack, W_o,
           k_cache, v_cache, _profile=False):
    in_maps, pos, winners = _prep_inputs(
        input_pos, hidden_states, attention_mask, W_pack, W_o, k_cache, v_cache)
    nc = _get_program(pos, winners)
    res = run_bass_kernel_spmd(nc, in_maps, list(range(NCORES)), trace=_profile)
    out = np.zeros((Q, HIDDEN), dtype=np.float64)
    for r in res.results:
        out += r["out"].astype(np.float64)
    full = out.astype(np.float32).reshape(1, Q, HIDDEN)
    if _profile:
        return full, res
    return full
